# revision 1
# baseline (speedup 1.0000x reference)
"""A-Connect conv kernel for TRN2, data-parallel over batch on 8 NeuronCores.

Computation (per sample b):
    Z[b] = conv2d(X[b], W * Werr[b], SAME) + bias * Berr[b]; out = relu(Z)

Mapping: batch 32 -> 4 samples per core. Per sample the conv is 9
shifted matmuls accumulated in PSUM. The output is computed in the
zero-padded spatial geometry (64 rows x 66 cols = 4224 = 33 tiles of
128 positions): in that flattened geometry the stationary operand for
tap (dy, dx) is a single stride-1 run of the padded input at offset
q0 + dy*66 + dx, which satisfies the BIR rule that matmul operand APs
have one free dimension. The two junk columns (xp = 0, 65) are sliced
away on the host. PSUM/output tiles are [spatial, F], matching NHWC so
stores are contiguous; the per-sample bias is added on the vector
engine in PSUM and relu+copyout runs on the scalar engine. A burst of
dependency-free warmup matmuls at kernel start releases the PE HAM
clock gate while the first input DMAs are in flight. Inputs are
pre-transposed/padded on host and fed in bf16 (measured rel err vs the
fp32 reference: 2.3e-3).
"""

import numpy as np
import ml_dtypes

B, H, Wd, Cin, F, KH, KW = 32, 64, 64, 128, 256, 3, 3
NCORES = 8
BPC = B // NCORES  # samples per core
HP, WP = H + 2, Wd + 2  # zero-padded spatial
NQ = H * WP  # padded output positions per sample: 64*66 = 4224
MT = NQ // 128  # 33 M-tiles of 128 padded positions
XLEN = 4368  # 1 lead zero + 66*66 flat padded image + tail zeros
# X arrives in two overlapping chunks so matmuls can start before the
# whole image is resident: tiles 0..16 read [0, 2448); tiles 17..32
# read [2176, 4368)
XSPLIT_MT = 17
XA_END = XSPLIT_MT * 128 + 134 + 128 + 10  # 2448
XB_OFF = XSPLIT_MT * 128  # 2176

_compiled = None  # cached Bass program so repeated kernel() calls reuse it


def _build_bass():
    from concourse import bacc, tile, mybir

    nc = bacc.Bacc("TRN2", target_bir_lowering=False, debug=False)
    bf16 = mybir.dt.bfloat16
    f32 = mybir.dt.float32

    xp = nc.dram_tensor("xp", [BPC, Cin, XLEN], bf16, kind="ExternalInput")
    wm = nc.dram_tensor("wm", [BPC, Cin, KH * KW, F], bf16, kind="ExternalInput")
    mb = nc.dram_tensor("mb", [BPC, 128, F], f32, kind="ExternalInput")
    y = nc.dram_tensor("y", [BPC, MT, 128, F], f32, kind="ExternalOutput")

    with tile.TileContext(nc) as tc:
        with (
            tc.tile_pool(name="xpool", bufs=2) as xpool,
            tc.tile_pool(name="wpool", bufs=2) as wpool,
            tc.tile_pool(name="bpool", bufs=2) as bpool,
            tc.tile_pool(name="opool", bufs=8) as opool,
            tc.tile_pool(name="cpool", bufs=1) as cpool,
            tc.tile_pool(name="pspool", bufs=7, space="PSUM") as pspool,
            tc.tile_pool(name="wupool", bufs=1, space="PSUM") as wupool,
        ):
            # PE warmup: ~6.4us of dependency-free matmuls. The HAM
            # clock-gate window is free-running, so worst case needs
            # ~6.8us of sustained PE activity before release; shorter
            # warmups make the real stream start cold (measured +2us)
            wu_in = cpool.tile([128, 512], bf16)
            nc.vector.memset(wu_in[:], 0.0)
            wu_ps = wupool.tile([128, 512], f32)
            for i in range(20):
                nc.tensor.matmul(
                    wu_ps[:],
                    wu_in[:, :128],
                    wu_in[:],
                    start=(i == 0),
                    stop=(i == 19),
                )
            for b in range(BPC):
                wt = wpool.tile([Cin, KH * KW, F], bf16)
                nc.sync.dma_start(wt[:], wm[b])
                xta = xpool.tile([Cin, XA_END], bf16, tag="xta")
                nc.sync.dma_start(xta[:], xp[b, :, :XA_END])
                bt = bpool.tile([128, F], f32)
                nc.sync.dma_start(bt[:], mb[b])
                xtb = xpool.tile([Cin, XLEN - XB_OFF], bf16, tag="xtb")
                nc.sync.dma_start(xtb[:], xp[b, :, XB_OFF:])
                for m in range(MT):
                    q0 = m * 128
                    ps = pspool.tile([128, F], f32)
                    for t in range(KH * KW):
                        dy, dx = t // KW, t % KW
                        off = q0 + dy * WP + dx
                        if m < XSPLIT_MT:
                            lhsT = xta[:, off : off + 128]
                        else:
                            lhsT = xtb[:, off - XB_OFF : off - XB_OFF + 128]
                        nc.tensor.matmul(
                            ps[:],
                            lhsT,
                            wt[:, t, :],
                            start=(t == 0),
                            stop=(t == 8),
                        )
                    # bias add on DVE (in PSUM), relu+copyout on ScalarE
                    nc.vector.tensor_add(ps[:], ps[:], bt[:])
                    ot = opool.tile([128, F], f32)
                    nc.scalar.activation(
                        ot[:], ps[:], mybir.ActivationFunctionType.Relu
                    )
                    nc.sync.dma_start(y[b, m], ot[:])
    nc.compile()
    return nc


def _prep_inputs(X, W, bias, Werr, Berr):
    bf16 = ml_dtypes.bfloat16
    X, W, bias, Werr, Berr = (
        np.asarray(a) for a in (X, W, bias, Werr, Berr)
    )
    # per-sample perturbed kernels, laid out [B, Cin, tap, F]
    memW = (W[None] * Werr).transpose(0, 3, 1, 2, 4).reshape(B, Cin, KH * KW, F)
    memW = np.ascontiguousarray(memW, dtype=bf16)
    # padded image, flattened with one lead zero so all tap offsets are >= 0
    Xpad = np.zeros((B, Cin, HP, WP), dtype=bf16)
    Xpad[:, :, 1 : H + 1, 1 : Wd + 1] = X.transpose(0, 3, 1, 2)
    Xp = np.zeros((B, Cin, XLEN), dtype=bf16)
    Xp[:, :, 1 : 1 + HP * WP] = Xpad.reshape(B, Cin, HP * WP)
    # bias broadcast across the 128 spatial partitions of an output tile
    mbias = (bias[None] * Berr).astype(np.float32)  # [B, F]
    mbias = np.ascontiguousarray(
        np.broadcast_to(mbias[:, None, :], (B, 128, F))
    )
    return Xp, memW, mbias


def _postprocess(y_cores):
    # y per core: [BPC, MT, 128, F] over padded positions (64 x 66);
    # drop the junk columns xp=0 and xp=65
    out = np.concatenate(y_cores, axis=0)  # [B, MT, 128, F]
    out = out.reshape(B, H, WP, F)[:, :, 1 : Wd + 1, :]
    return np.ascontiguousarray(out)


def kernel(X, W, bias, Werr, Berr):
    global _compiled
    from concourse.bass_utils import run_bass_kernel_spmd

    if _compiled is None:
        _compiled = _build_bass()
    nc = _compiled

    Xp, memW, mbias = _prep_inputs(X, W, bias, Werr, Berr)
    in_maps = [
        {
            "xp": Xp[c * BPC : (c + 1) * BPC],
            "wm": memW[c * BPC : (c + 1) * BPC],
            "mb": mbias[c * BPC : (c + 1) * BPC],
        }
        for c in range(NCORES)
    ]
    res = run_bass_kernel_spmd(nc, in_maps, core_ids=list(range(NCORES)))
    return _postprocess([r["y"] for r in res.results])



# revision 3
# speedup vs baseline: 1.0401x; 1.0401x over previous
"""A-Connect conv kernel for TRN2, data-parallel over batch on 8 NeuronCores.

Computation (per sample b):
    Z[b] = conv2d(X[b], W * Werr[b], SAME) + bias * Berr[b]; out = relu(Z)

Mapping: batch 32 -> 4 samples per core. The 3x3 conv is decomposed with
1D Winograd F(2,3) along the width axis, which cuts tensor-engine work
to 2/3 of direct convolution (the direct kernel is matmul-stream bound):
for each Winograd position u in 0..3 the device computes

    m_u[f, (y, tx)] = sum_dy sum_cin U[u, dy, cin, f] * V_u[cin, y+dy-1, tx]

as 3 PSUM-accumulated matmuls with N=512 moving streams (the [Cin, 512]
moving operand is a contiguous run of V_u because V rows are 32 wide).
Both Winograd transforms are linear data prep and run on the host:
V_u = B^T d (stride-1 column combos of the zero-padded input) before the
kernel, and z0 = m0+m1+m2, z1 = m1-m2-m3 plus bias and relu after it.
Output layout is [F, positions] so every store is contiguous; the host
transposes back to NHWC. PSUM->SBUF evacuation (with bf16 downcast) is
split between the scalar and vector engines so neither stalls the PE.
A burst of dependency-free warmup matmuls at kernel start releases the
PE HAM clock gate while the first input DMAs are in flight. Measured
rel err vs the fp32 reference: ~1e-3 (bf16 operands + bf16 m storage).
"""

import numpy as np
import ml_dtypes

B, H, Wd, Cin, F, KH, KW = 32, 64, 64, 128, 256, 3, 3
NCORES = 8
BPC = B // NCORES  # samples per core
NU = 4  # Winograd F(2,3) positions
NDY = 3  # row taps
NTX = Wd // 2  # 32 column tiles (2 output cols each)
VR = H + 2  # V rows cover padded input rows -1..64
VLEN = VR * NTX  # 2112
NC = 4  # position chunks of 512 (16 y-rows) per (fh, u)
NFH = 2  # halves of F on the 128 output partitions
VHEAD = (16 + 2) * NTX + 512 - NTX * 16  # first-chunk V prefix: 18 rows = 576

_compiled = None  # cached Bass program so repeated kernel() calls reuse it


def _build_bass():
    from concourse import bacc, tile, mybir

    nc = bacc.Bacc("TRN2", target_bir_lowering=False, debug=False)
    bf16 = mybir.dt.bfloat16
    f32 = mybir.dt.float32

    vts = nc.dram_tensor("vts", [BPC, NU, Cin, VLEN], bf16, kind="ExternalInput")
    wu = nc.dram_tensor("wu", [BPC, Cin, NU * NDY * NFH, 128], bf16, kind="ExternalInput")
    y = nc.dram_tensor("y", [BPC, NFH * NC * NU, 128, 512], bf16, kind="ExternalOutput")

    with tile.TileContext(nc) as tc:
        with (
            tc.tile_pool(name="vpool", bufs=2) as vpool,
            tc.tile_pool(name="wpool", bufs=2) as wpool,
            tc.tile_pool(name="opool", bufs=8) as opool,
            tc.tile_pool(name="cpool", bufs=1) as cpool,
            tc.tile_pool(name="pspool", bufs=7, space="PSUM") as pspool,
            tc.tile_pool(name="wupool", bufs=1, space="PSUM") as wupool,
        ):
            # PE warmup: dependency-free matmuls release the HAM clock
            # gate (~3.4us of sustained activity) while the first input
            # DMAs land.
            wu_in = cpool.tile([128, 512], bf16)
            nc.vector.memset(wu_in[:], 0.0)
            wu_ps = wupool.tile([128, 512], f32)
            for i in range(18):
                nc.tensor.matmul(
                    wu_ps[:],
                    wu_in[:, :128],
                    wu_in[:],
                    start=(i == 0),
                    stop=(i == 17),
                )
            evac = 0
            for b in range(BPC):
                wt = wpool.tile([Cin, NU * NDY * NFH, 128], bf16)
                nc.sync.dma_start(wt[:], wu[b])
                vt = []
                for u in range(NU):
                    t = vpool.tile([Cin, VLEN], bf16, tag=f"v{u}")
                    # head first so chunk 0 can start before the tail lands
                    nc.sync.dma_start(t[:, :VHEAD], vts[b, u, :, :VHEAD])
                    vt.append(t)
                for u in range(NU):
                    nc.sync.dma_start(vt[u][:, VHEAD:], vts[b, u, :, VHEAD:])
                for c in range(NC):
                    for fh in range(NFH):
                        for u in range(NU):
                            ps = pspool.tile([128, 512], f32)
                            for dy in range(NDY):
                                off = (c * 16 + dy) * NTX
                                nc.tensor.matmul(
                                    ps[:],
                                    wt[:, (u * NDY + dy) * NFH + fh, :],
                                    vt[u][:, off : off + 512],
                                    start=(dy == 0),
                                    stop=(dy == NDY - 1),
                                )
                            ot = opool.tile([128, 512], bf16)
                            # alternate evacuation engine so each stays
                            # well under the PE's pace
                            if evac % 2 == 0:
                                nc.scalar.activation(
                                    ot[:], ps[:], mybir.ActivationFunctionType.Copy
                                )
                            else:
                                nc.vector.tensor_copy(ot[:], ps[:])
                            evac += 1
                            nc.sync.dma_start(y[b, (fh * NC + c) * NU + u], ot[:])
    nc.compile()
    return nc


def _prep_inputs(X, W, bias, Werr, Berr):
    bf16 = ml_dtypes.bfloat16
    X, W, bias, Werr, Berr = (np.asarray(a) for a in (X, W, bias, Werr, Berr))
    # Winograd weight transform along dx: U[u] = G @ g, then per-sample
    # perturbed kernels laid out for [Cin, u*dy*fh, 128] stationary slices
    G = np.array(
        [[1, 0, 0], [0.5, 0.5, 0.5], [0.5, -0.5, 0.5], [0, 0, 1]], np.float32
    )
    memW = W[None] * Werr  # [B, dy, dx, Cin, F]
    U = np.einsum("ux,byxcf->bcuyf", G, memW)  # [B, Cin, u, dy, F]
    wu = U.reshape(B, Cin, NU, NDY, NFH, 128).reshape(B, Cin, NU * NDY * NFH, 128)
    wu = np.ascontiguousarray(wu, dtype=bf16)
    # Winograd input transform: V_u = B^T d over even/odd padded columns
    Xpad = np.zeros((B, Cin, H + 2, Wd + 2), np.float32)
    Xpad[:, :, 1 : H + 1, 1 : Wd + 1] = X.transpose(0, 3, 1, 2)
    d = [Xpad[:, :, :, j : j + 2 * NTX : 2] for j in range(4)]  # [B,Cin,66,32] each
    V = np.stack(
        [d[0] - d[2], d[1] + d[2], d[2] - d[1], d[1] - d[3]], axis=1
    )  # [B, u, Cin, 66, 32]
    vts = np.ascontiguousarray(V.reshape(B, NU, Cin, VLEN), dtype=bf16)
    return vts, wu


def _postprocess(y_cores, bias, Berr):
    m = np.concatenate(y_cores, axis=0).astype(np.float32)  # [B, fh*c*u, 128, 512]
    m = m.reshape(B, NFH, NC, NU, 128, 16, NTX)
    # -> [B, u, y(c*16+yy), tx, f(fh*128+p)]
    m = m.transpose(0, 3, 2, 5, 6, 1, 4).reshape(B, NU, H, NTX, F)
    z0 = m[:, 0] + m[:, 1] + m[:, 2]
    z1 = m[:, 1] - m[:, 2] - m[:, 3]
    out = np.empty((B, H, Wd, F), np.float32)
    out[:, :, 0::2, :] = z0
    out[:, :, 1::2, :] = z1
    out += (np.asarray(bias)[None] * np.asarray(Berr))[:, None, None, :]
    np.maximum(out, 0.0, out=out)
    return out


def kernel(X, W, bias, Werr, Berr):
    global _compiled
    from concourse.bass_utils import run_bass_kernel_spmd

    if _compiled is None:
        _compiled = _build_bass()
    nc = _compiled

    vts, wu = _prep_inputs(X, W, bias, Werr, Berr)
    in_maps = [
        {
            "vts": vts[c * BPC : (c + 1) * BPC],
            "wu": wu[c * BPC : (c + 1) * BPC],
        }
        for c in range(NCORES)
    ]
    res = run_bass_kernel_spmd(nc, in_maps, core_ids=list(range(NCORES)))
    return _postprocess([r["y"] for r in res.results], bias, Berr)


# revision 7
# speedup vs baseline: 1.3028x; 1.2526x over previous
"""A-Connect conv kernel for TRN2, data-parallel over batch on 8 NeuronCores.

Computation (per sample b):
    Z[b] = conv2d(X[b], W * Werr[b], SAME) + bias * Berr[b]; out = relu(Z)

Mapping: batch 32 -> 4 samples per core. The 3x3 conv is decomposed with
1D Winograd F(2,3) along the width axis, which cuts tensor-engine work
to 2/3 of direct convolution (the direct kernel is matmul-stream bound):
for each Winograd position u in 0..3 the device computes

    m_u[f, (y, tx)] = sum_dy sum_cin U[u, dy, cin, f] * V_u[cin, y+dy-1, tx]

as 3 PSUM-accumulated matmuls with N=512 moving streams (the [Cin, 512]
moving operand is a contiguous run of V_u because V rows are 32 wide).
The Winograd input transform V_u = B^T d is linear data prep and runs on
the host (stride-1 column combos of the zero-padded input). The output
transform is split: the vector engine folds z0 = m0+m1+m2 and t1 = m1-m2
while evacuating PSUM (3 tensor_tensor ops per group, under the PE's
pace), the scalar engine evacuates m3, and the host finishes
z1 = t1 - m3 plus bias and relu. This ships 3 bf16 tiles per group
instead of 4, keeping total HBM traffic under the matmul time.

Queue discipline: input DMAs ride the gpsimd (SWDGE) queue prefetched
one sample ahead, output DMAs ride the sync queue, so neither convoys
behind the other (the v2 kernel lost ~40us to PE stalls + HAM
re-throttling from exactly that convoy). A burst of dependency-free
warmup matmuls at kernel start releases the PE HAM clock gate while the
first input DMAs are in flight. Measured rel err vs the fp32 reference:
~3.7e-3 (bf16 operands + bf16 m storage).
"""

import numpy as np
import ml_dtypes

B, H, Wd, Cin, F, KH, KW = 32, 64, 64, 128, 256, 3, 3
NCORES = 8
BPC = B // NCORES  # samples per core
NU = 4  # Winograd F(2,3) positions
NDY = 3  # row taps
NTX = Wd // 2  # 32 column tiles (2 output cols each)
VR = H + 2  # V rows cover padded input rows -1..64
VLEN = VR * NTX  # 2112
NC = 4  # position chunks of 512 (16 y-rows) per (fh, u)
NFH = 2  # halves of F on the 128 output partitions
NK = 3  # output tiles shipped per group: z0', t1, m3

_compiled = None  # cached Bass program so repeated kernel() calls reuse it


def _build_bass():
    from concourse import bacc, tile, mybir

    nc = bacc.Bacc("TRN2", target_bir_lowering=False, debug=False)
    bf16 = mybir.dt.bfloat16
    f32 = mybir.dt.float32
    add = mybir.AluOpType.add
    sub = mybir.AluOpType.subtract

    vts = nc.dram_tensor("vts", [BPC, Cin, NU, VLEN], bf16, kind="ExternalInput")
    wu = nc.dram_tensor("wu", [BPC, Cin, NU * NDY * NFH, 128], bf16, kind="ExternalInput")
    y = nc.dram_tensor("y", [BPC, NFH * NC * NK, 128, 512], bf16, kind="ExternalOutput")

    with tile.TileContext(nc) as tc:
        with (
            tc.tile_pool(name="vpool", bufs=2) as vpool,
            tc.tile_pool(name="wpool", bufs=2) as wpool,
            tc.tile_pool(name="opool", bufs=12) as opool,
            tc.tile_pool(name="tpool", bufs=4) as tpool,
            tc.tile_pool(name="cpool", bufs=1) as cpool,
            tc.tile_pool(name="pspool", bufs=7, space="PSUM") as pspool,
            tc.tile_pool(name="wupool", bufs=1, space="PSUM") as wupool,
        ):
            # PE warmup: dependency-free matmuls release the HAM clock
            # gate (~3.4us of sustained activity) while the first input
            # DMAs land.
            wu_in = cpool.tile([128, 512], bf16)
            nc.vector.memset(wu_in[:], 0.0)
            wu_ps = wupool.tile([128, 512], f32)
            for i in range(18):
                nc.tensor.matmul(
                    wu_ps[:],
                    wu_in[:, :128],
                    wu_in[:],
                    start=(i == 0),
                    stop=(i == 17),
                )

            vt = [None] * BPC
            wt = [None] * BPC

            def prefetch(b):
                # inputs ride the gpsimd (SWDGE) queue so they never
                # convoy behind output DMAs on the sync queue
                vt[b] = vpool.tile([Cin, NU, VLEN], bf16, name="vt")
                nc.gpsimd.dma_start(vt[b][:], vts[b])
                wt[b] = wpool.tile([Cin, NU * NDY * NFH, 128], bf16, name="wt")
                nc.gpsimd.dma_start(wt[b][:], wu[b])

            prefetch(0)
            for b in range(BPC):
                if b + 1 < BPC:
                    prefetch(b + 1)
                for c in range(NC):
                    for fh in range(NFH):
                        ms = []
                        for u in range(NU):
                            ps = pspool.tile([128, 512], f32)
                            for dy in range(NDY):
                                off = (c * 16 + dy) * NTX
                                nc.tensor.matmul(
                                    ps[:],
                                    wt[b][:, (u * NDY + dy) * NFH + fh, :],
                                    vt[b][:, u, off : off + 512],
                                    start=(dy == 0),
                                    stop=(dy == NDY - 1),
                                )
                            ms.append(ps)
                        base = (fh * NC + c) * NK
                        # fold z0' = m0+m1+m2 and t1 = m1-m2 while
                        # evacuating PSUM. The DVE has a single PSUM read
                        # port, so every op pairs one PSUM operand with
                        # one SBUF operand: ScalarE stages m2 into SBUF
                        # first, then the DVE chains through it.
                        a = tpool.tile([128, 512], f32, tag="a")
                        nc.scalar.activation(
                            a[:], ms[2][:], mybir.ActivationFunctionType.Copy
                        )
                        bb = tpool.tile([128, 512], f32, tag="bb")
                        nc.vector.tensor_tensor(bb[:], ms[1][:], a[:], add)
                        z0 = opool.tile([128, 512], bf16, tag="z0")
                        nc.vector.tensor_tensor(z0[:], ms[0][:], bb[:], add)
                        t1 = opool.tile([128, 512], bf16, tag="t1")
                        nc.vector.tensor_tensor(t1[:], ms[1][:], a[:], sub)
                        m3 = opool.tile([128, 512], bf16, tag="m3")
                        nc.scalar.activation(
                            m3[:], ms[3][:], mybir.ActivationFunctionType.Copy
                        )
                        nc.sync.dma_start(y[b, base + 0], z0[:])
                        nc.sync.dma_start(y[b, base + 1], t1[:])
                        nc.sync.dma_start(y[b, base + 2], m3[:])
    nc.compile()
    return nc


def _prep_inputs(X, W, bias, Werr, Berr):
    bf16 = ml_dtypes.bfloat16
    X, W, bias, Werr, Berr = (np.asarray(a) for a in (X, W, bias, Werr, Berr))
    # Winograd weight transform along dx: U[u] = G @ g, then per-sample
    # perturbed kernels laid out for [Cin, u*dy*fh, 128] stationary slices
    G = np.array(
        [[1, 0, 0], [0.5, 0.5, 0.5], [0.5, -0.5, 0.5], [0, 0, 1]], np.float32
    )
    memW = W[None] * Werr  # [B, dy, dx, Cin, F]
    U = np.einsum("ux,byxcf->bcuyf", G, memW)  # [B, Cin, u, dy, F]
    wu = U.reshape(B, Cin, NU, NDY, NFH, 128).reshape(B, Cin, NU * NDY * NFH, 128)
    wu = np.ascontiguousarray(wu, dtype=bf16)
    # Winograd input transform: V_u = B^T d over even/odd padded columns
    Xpad = np.zeros((B, Cin, H + 2, Wd + 2), np.float32)
    Xpad[:, :, 1 : H + 1, 1 : Wd + 1] = X.transpose(0, 3, 1, 2)
    d = [Xpad[:, :, :, j : j + 2 * NTX : 2] for j in range(4)]  # [B,Cin,66,32] each
    V = np.stack(
        [d[0] - d[2], d[1] + d[2], d[2] - d[1], d[1] - d[3]], axis=2
    )  # [B, Cin, u, 66, 32]
    vts = np.ascontiguousarray(V.reshape(B, Cin, NU, VLEN), dtype=bf16)
    return vts, wu


def _postprocess(y_cores, bias, Berr):
    m = np.concatenate(y_cores, axis=0).astype(np.float32)  # [B, fh*c*k, 128, 512]
    m = m.reshape(B, NFH, NC, NK, 128, 16, NTX)
    # -> [B, k, y(c*16+yy), tx, f(fh*128+p)]
    m = m.transpose(0, 3, 2, 5, 6, 1, 4).reshape(B, NK, H, NTX, F)
    z0 = m[:, 0]
    z1 = m[:, 1] - m[:, 2]  # t1 - m3
    out = np.empty((B, H, Wd, F), np.float32)
    out[:, :, 0::2, :] = z0
    out[:, :, 1::2, :] = z1
    out += (np.asarray(bias)[None] * np.asarray(Berr))[:, None, None, :]
    np.maximum(out, 0.0, out=out)
    return out


def kernel(X, W, bias, Werr, Berr):
    global _compiled
    from concourse.bass_utils import run_bass_kernel_spmd

    if _compiled is None:
        _compiled = _build_bass()
    nc = _compiled

    vts, wu = _prep_inputs(X, W, bias, Werr, Berr)
    in_maps = [
        {
            "vts": vts[c * BPC : (c + 1) * BPC],
            "wu": wu[c * BPC : (c + 1) * BPC],
        }
        for c in range(NCORES)
    ]
    res = run_bass_kernel_spmd(nc, in_maps, core_ids=list(range(NCORES)))
    return _postprocess([r["y"] for r in res.results], bias, Berr)
